# revision 27
# baseline (speedup 1.0000x reference)
"""Trainium2 Bass kernel for DecoderAttn ('general' attention score + softmax).

Reference computation (per batch b):
    energies[t] = dec[b] . (enc[b,t] @ W.T + bias)
    attn = softmax(energies over t)

Algebraic rewrite used here:
    energies[t] = enc[b,t] . (W.T @ dec[b])  +  (bias . dec[b])
The second term is constant over t, so it drops out of the softmax exactly.
This turns an O(B*T*H^2) matmul chain into an O(B*H^2 + B*T*H) streaming
problem: precompute v[b] = W.T @ dec[b] = (dec @ W)[b] on the tensor engine,
then a fused multiply+reduce (DVE tensor_tensor_reduce) over the encoder
stream, then a tiny softmax.

Sharding: data-parallel over batch B=32 across 8 NeuronCores (4 batches per
core); W replicated.
"""

import numpy as np
from contextlib import ExitStack

import concourse.bass as bass
import concourse.tile as tile
from concourse import bacc, mybir, masks
from concourse.bass_utils import run_bass_kernel_spmd

F32 = mybir.dt.float32

B, T, H = 32, 2048, 1024
NCORES = 8
BL = B // NCORES           # batches per core
TCH = T // 128             # 128-row t-chunks per batch
OCH = H // 128             # 128-row o-chunks of W


def build_kernel(bl=BL, t=T, h=H, enc_bufs=16, repeat=1, scr_bufs=4):
    tch = t // 128
    och = h // 128
    nhh = h // 512  # 512-wide halves of the H free dim for matmul N-limit

    nc = bacc.Bacc("TRN2", target_bir_lowering=False, debug=False)

    dec = nc.dram_tensor("dec", [bl, h], F32, kind="ExternalInput")
    enc = nc.dram_tensor("enc", [bl, t, h], F32, kind="ExternalInput")
    w = nc.dram_tensor("w", [h, h], F32, kind="ExternalInput")
    attn = nc.dram_tensor("attn", [bl, t], F32, kind="ExternalOutput")

    with tile.TileContext(nc) as tc, ExitStack() as ctx:
        const = ctx.enter_context(tc.tile_pool(name="const", bufs=1))
        wpool = ctx.enter_context(tc.tile_pool(name="wpool", bufs=1))
        encp = ctx.enter_context(tc.tile_pool(name="encp", bufs=enc_bufs))
        scr = ctx.enter_context(tc.tile_pool(name="scr", bufs=scr_bufs))
        sm = ctx.enter_context(tc.tile_pool(name="sm", bufs=2))
        outp = ctx.enter_context(tc.tile_pool(name="outp", bufs=2))
        psA = ctx.enter_context(tc.tile_pool(name="psA", bufs=2, space="PSUM"))
        psS = ctx.enter_context(tc.tile_pool(name="psS", bufs=3, space="PSUM"))

        # ---- constants ----
        ident = const.tile([128, 128], F32)
        masks.make_identity(nc, ident[:])
        ones = const.tile([1, 128], F32)
        nc.gpsimd.memset(ones[:], 1.0)

        # long-lived state
        epool = ctx.enter_context(tc.tile_pool(name="epool", bufs=2))
        vb_all = const.tile([128, bl * h], F32)       # v[b] broadcast to 128 parts
        dec_sb = const.tile([bl, h], F32)
        decT = const.tile([128, och * bl], F32)       # chunk oc at cols [oc*bl, (oc+1)*bl)
        v_sb = const.tile([1, bl * h], F32)           # all rows on partition 0

        # ---- phase 1: v = dec @ W  (tensor engine) ----
        nc.sync.dma_start(dec_sb[:], dec[:, :])

        for oc in range(och):
            dT_ps = psS.tile([128, bl], F32, tag="small")
            nc.tensor.transpose(
                dT_ps[:], dec_sb[:, oc * 128:(oc + 1) * 128], ident[0:bl, 0:bl]
            )
            nc.vector.tensor_copy(decT[:, oc * bl:(oc + 1) * bl], dT_ps[:])

        w_tiles = []
        for oc in range(och):
            wt = wpool.tile([128, h], F32, tag=f"w{oc}")
            nc.sync.dma_start(wt[:], w[oc * 128:(oc + 1) * 128, :])
            w_tiles.append(wt)

        # v then broadcast, batch-at-a-time so batch 0's stream can start early
        for b in range(bl):
            for hh in range(nhh):
                v_ps = psA.tile([1, 512], F32, tag="work")
                for oc in range(och):
                    nc.tensor.matmul(
                        v_ps[:],
                        decT[:, oc * bl + b: oc * bl + b + 1],
                        w_tiles[oc][:, hh * 512:(hh + 1) * 512],
                        start=(oc == 0),
                        stop=(oc == och - 1),
                    )
                nc.vector.tensor_copy(
                    v_sb[:, b * h + hh * 512: b * h + (hh + 1) * 512], v_ps[:]
                )
                # broadcast v[b] across all 128 partitions
                vb_ps = psA.tile([128, 512], F32, tag="work")
                nc.tensor.matmul(
                    vb_ps[:],
                    ones[0:1, 0:128],
                    v_sb[0:1, b * h + hh * 512: b * h + (hh + 1) * 512],
                    start=True,
                    stop=True,
                )
                nc.scalar.copy(
                    vb_all[:, b * h + hh * 512: b * h + (hh + 1) * 512], vb_ps[:]
                )

        # ---- phase 3+4: stream encoder, fused dot, softmax ----
        for _rep in range(repeat):
            _phase34(nc, tc, bl, t, h, tch, enc, attn, encp, scr, sm, outp, psS,
                     epool, vb_all, ones, ident)

    nc.compile()
    return nc


def _phase34(nc, tc, bl, t, h, tch, enc, attn, encp, scr, sm, outp, psS,
             epool, vb_all, ones, ident):
    if True:
        for b in range(bl):
            vb = vb_all[:, b * h:(b + 1) * h]
            e_t = epool.tile([128, tch], F32, tag=f"e{b}")
            for tcix in range(tch):
                et = encp.tile([128, h], F32, tag="enc")
                nc.sync.dma_start(et[:], enc[b, tcix * 128:(tcix + 1) * 128, :])
                sc = scr.tile([128, h], F32, tag="scr")
                nc.vector.tensor_mul(sc[:], et[:], vb)
                dump = scr.tile([128, h], F32, tag="dump")
                nc.scalar.activation(
                    dump[:], sc[:], mybir.ActivationFunctionType.Copy,
                    bias=0.0, scale=1.0,
                    accum_out=e_t[:, tcix: tcix + 1],
                )

            # softmax over the [128, tch] energies of this batch
            e_b = e_t[:, :]

            m1 = sm.tile([128, 1], F32, tag="m1")
            nc.vector.tensor_reduce(
                out=m1[:], in_=e_b, axis=mybir.AxisListType.X, op=mybir.AluOpType.max
            )
            m1T = psS.tile([1, 128], F32, tag="small")
            nc.tensor.transpose(m1T[:], m1[:], ident[:, :])
            M = sm.tile([1, 1], F32, tag="M")
            nc.vector.tensor_reduce(
                out=M[:], in_=m1T[0:1, :], axis=mybir.AxisListType.X,
                op=mybir.AluOpType.max,
            )
            Mb_ps = psS.tile([128, 1], F32, tag="small")
            nc.tensor.matmul(Mb_ps[:], ones[0:1, 0:128], M[0:1, 0:1],
                             start=True, stop=True)
            negM = sm.tile([128, 1], F32, tag="negM")
            nc.scalar.mul(negM[:], Mb_ps[:], -1.0)

            p_b = sm.tile([128, tch], F32, tag="p")
            s1 = sm.tile([128, 1], F32, tag="s1")
            nc.scalar.activation(
                p_b[:], e_b, mybir.ActivationFunctionType.Exp,
                bias=negM[:, 0:1], scale=1.0, accum_out=s1[:],
            )
            s1T = psS.tile([1, 128], F32, tag="small")
            nc.tensor.transpose(s1T[:], s1[:], ident[:, :])
            S = sm.tile([1, 1], F32, tag="S")
            nc.vector.tensor_reduce(
                out=S[:], in_=s1T[0:1, :], axis=mybir.AxisListType.X,
                op=mybir.AluOpType.add,
            )
            R = sm.tile([1, 1], F32, tag="R")
            nc.vector.reciprocal(R[:], S[:])
            Rb_ps = psS.tile([128, 1], F32, tag="small")
            nc.tensor.matmul(Rb_ps[:], ones[0:1, 0:128], R[0:1, 0:1],
                             start=True, stop=True)
            Rb = sm.tile([128, 1], F32, tag="Rbs")
            nc.scalar.copy(Rb[:], Rb_ps[:])

            a_b = sm.tile([128, tch], F32, tag="a")
            nc.vector.tensor_scalar_mul(a_b[:], p_b[:], Rb[:, 0:1])

            aT_ps = psS.tile([tch, 128], F32, tag="small")
            nc.tensor.transpose(aT_ps[:], a_b[:], ident[:, :])
            aT = outp.tile([tch, 128], F32, tag="aTs")
            nc.vector.tensor_copy(aT[:], aT_ps[:])
            nc.sync.dma_start(
                attn[b].rearrange("(c p) -> c p", p=128), aT[:]
            )


_NC_CACHE = {}


def _get_nc():
    if "nc" not in _NC_CACHE:
        _NC_CACHE["nc"] = build_kernel()
    return _NC_CACHE["nc"]


def run_sharded(decoder_output, encoder_outputs, W, trace=False, **kw):
    nc = _get_nc()
    in_maps = []
    for c in range(NCORES):
        sl = slice(c * BL, (c + 1) * BL)
        in_maps.append({
            "dec": np.ascontiguousarray(decoder_output[sl], dtype=np.float32),
            "enc": np.ascontiguousarray(encoder_outputs[sl], dtype=np.float32),
            "w": np.ascontiguousarray(W, dtype=np.float32),
        })
    res = run_bass_kernel_spmd(nc, in_maps, list(range(NCORES)), trace=trace, **kw)
    attn = np.concatenate([res.results[c]["attn"] for c in range(NCORES)], axis=0)
    return attn, res


def kernel(decoder_output, encoder_outputs, W, b=None, **_unused):
    # b (the Linear bias) shifts every energy of a batch equally -> cancels in
    # softmax; it is deliberately unused.
    attn, _ = run_sharded(decoder_output, encoder_outputs, W)
    return attn.reshape(B, T, 1).astype(np.float32)
